# revision 13
# baseline (speedup 1.0000x reference)
"""Causal self-attention on 8 Trainium2 NeuronCores.

Sharding: B*H = 2*12 = 24 (batch, head) pairs -> 3 heads per core.
Core i handles batch i//4, heads 3*(i%4) .. 3*(i%4)+2.
Each core computes q/k/v projections for its 3 heads (tensor-parallel slice
of wq/wk/wv), causal attention, and a partial out-projection against its
192 columns of wo. Host sums the 4 partials per batch (the "all-reduce").

v2 (vs. the original baseline):
  - x is passed PRE-TRANSPOSED (xT [768, T]) and in bf16: no PE transposes
    or PSUM->SBUF copies for x, and half the DMA bytes.
  - All matmul operands are bf16 (PSUM stays fp32): same PE rate as fp32r
    at free-dim>=256, but halves SBUF traffic and enables fast DVE modes.
  - Projections, v-transpose and attention are pipelined per 512-row block
    (attention for q-block qb needs k/v rows < 256*qb+256, which is exactly
    what the projection of the same block provides).
  - Engine rebalance: projection bias-add epilogue on DVE (tensor_scalar),
    masks stay DVE (bf16 fast mode), softmax epilogue identical to baseline
    (DVE reciprocal + PE K=1 broadcast matmul).

Per-core kernel:
  - qT/kT computed in [64, T] layout; v computed via vT then PE-transposed
    to natural [T, 64] with a ones column appended (for softmax denominators).
  - S_T[kblock, qblock] = K_blk @ Q_blk.T  (contraction d=64)
  - P_T = exp(S_T / 8)   on ACT, grouped over 4 kblocks
  - causal masking: multiply diagonal-crossing blocks by a 0/1 triangle mask
  - attnU_T[65, TQ] += Vaug_blk.T @ P_T   (row 64 = softmax denominator)
  - divide via reciprocal + partition-broadcast matmul, then
    y[T,768] partial = attnT.T @ woT_slice.
No max-subtraction in softmax: logits here have |.| <~ 2, exp is safe.

Partition-base alignment: per-head pairs live at the same partition offset:
  q01 [128,T] = qT_h0 (rows 0:64) | qT_h1 (rows 64:128)
  k01 [128,T] = kT_h0 | kT_h1
  qv0 [128,T] = qT_h2 | vT_h0
  kv1 [128,T] = kT_h2 | vT_h1
  v2t [64,T]  = vT_h2
"""

import numpy as np

import concourse.bass as bass
import concourse.mybir as mybir
from concourse import bacc
from concourse import tile
from concourse.bass_utils import run_bass_kernel_spmd
from concourse.masks import make_identity

F32 = mybir.dt.float32
F32R = mybir.dt.float32r
BF16 = mybir.dt.bfloat16

EMBED = 768
NHEAD = 12
DH = 64
B = 2
T = 4096
HPC = 3          # heads per core
CH = HPC * DH    # 192 channels per core
NCORES = 8


def build_program(t=T):
    """Build the single-core SPMD Bass program."""
    nqb = t // 256   # q blocks of 256
    ntb = t // 512   # projection T-blocks of 512

    nc = bacc.Bacc("TRN2", target_bir_lowering=False, debug=False,
                   num_devices=NCORES)

    xT_d = nc.dram_tensor("xT", [EMBED, t], BF16, kind="ExternalInput")
    # columns: q0,q1 | k0,k1 | q2,v0 | k2,v1 | v2   (64 each)
    wqkv_d = nc.dram_tensor("wqkvT", [EMBED, 576], BF16, kind="ExternalInput")
    bqkv_d = nc.dram_tensor("bqkv", [576, 1], F32, kind="ExternalInput")
    wo_d = nc.dram_tensor("woT", [CH, EMBED], BF16, kind="ExternalInput")
    y_d = nc.dram_tensor("y", [t, EMBED], F32, kind="ExternalOutput")

    Act = mybir.ActivationFunctionType

    with tile.TileContext(nc) as tc:
        with (
            tc.tile_pool(name="const", bufs=1) as cpool,
            tc.tile_pool(name="persist", bufs=1) as perm,
        ):
            ident = cpool.tile([128, 128], BF16, tag="ident")
            make_identity(nc, ident)
            # bigadd[si, u] = -240.0 if si > u - 128 else 0.0 (causal mask,
            # added to logits inside the S matmul accumulation group via
            # lhsT=identity: exp(0.125*(s-240)) ~ 9e-14 ~ 0).
            # diag kblock (d=0)  -> slice [:, 128:384];  d=-128 -> [:, 0:256]
            bigadd = cpool.tile([128, 384], BF16, tag="bigadd")
            nc.gpsimd.memset(bigadd, -240.0)
            nc.gpsimd.affine_select(
                out=bigadd, in_=bigadd,
                compare_op=mybir.AluOpType.is_ge, fill=0.0,
                base=127, pattern=[[-1, 384]], channel_multiplier=1,
            )

            # weights (tiles only; DMAs are emitted interleaved with the
            # first x-tile loads inside the tb=0 iteration)
            wqkv_sb = [cpool.tile([128, 576], BF16, name=f"wqkv{kt}",
                                  tag=f"wqkv{kt}") for kt in range(6)]
            bias_sb = [cpool.tile([128, 1], F32, name=f"bias{mc}",
                                  tag=f"bias{mc}") for mc in range(5)]
            wo_sb = [cpool.tile([64, EMBED], BF16, name=f"wo{h}",
                                tag=f"wo{h}") for h in range(3)]

            # persistent activations (bf16)
            q01 = perm.tile([128, t], BF16, tag="q01")
            k01 = perm.tile([128, t], BF16, tag="k01")
            qv0 = perm.tile([128, t], BF16, tag="qv0")
            kv1 = perm.tile([128, t], BF16, tag="kv1")
            v2t = perm.tile([64, t], BF16, tag="v2t")
            # all-ones row at partition 64 (for denominator broadcast mm)
            ones65 = cpool.tile([65, 64], F32R, tag="ones65")
            nc.gpsimd.memset(ones65.bitcast(F32), 1.0)
            # v natural, 65-wide per 128-row chunk (col 64 = ones)
            vs = [perm.tile([128, (t // 128) * 65], BF16, name=f"vs{h}",
                            tag=f"vs{h}")
                  for h in range(3)]
            for h in range(3):
                nc.gpsimd.memset(vs[h], 1.0)

            proj_dest = [q01, k01, qv0, kv1, v2t]

            def q_ap(h):
                return (q01[0:64], q01[64:128], qv0[0:64])[h]

            def k_ap(h):
                return (k01[0:64], k01[64:128], kv1[0:64])[h]

            v_src = [qv0[64:128], kv1[64:128], v2t[0:64]]
            v_idn = [ident[64:128, 64:128], ident[64:128, 64:128],
                     ident[0:64, 0:64]]

            with (
                tc.tile_pool(name="xtpool", bufs=2) as xtpool,
                tc.tile_pool(name="projpsum", bufs=1, space="PSUM") as projpsum,
                tc.tile_pool(name="spsum", bufs=2, space="PSUM") as spsum,
                tc.tile_pool(name="miscpsum", bufs=1, space="PSUM") as miscpsum,
                tc.tile_pool(name="ppool", bufs=4) as ppool,
                tc.tile_pool(name="apool", bufs=3) as apool,
                tc.tile_pool(name="rpool", bufs=4) as rpool,
                tc.tile_pool(name="ysb", bufs=3) as ysb,
            ):
                for tb in range(ntb):
                    # ---- projections for rows [tb*512, (tb+1)*512) ----
                    xts = [xtpool.tile([128, 512], BF16, tag=f"xt{ct}",
                                       name=f"xt{ct}_{tb}")
                           for ct in range(6)]
                    for ct in range(6):
                        if tb == 0:
                            nc.sync.dma_start(
                                wqkv_sb[ct],
                                wqkv_d[ct * 128:(ct + 1) * 128, :])
                        nc.sync.dma_start(
                            xts[ct],
                            xT_d[ct * 128:(ct + 1) * 128,
                                 tb * 512:(tb + 1) * 512])
                    if tb == 0:
                        for mc in range(5):
                            mw = 128 if mc < 4 else 64
                            nc.sync.dma_start(
                                bias_sb[mc][:mw, :],
                                bqkv_d[mc * 128:mc * 128 + mw, :])
                        for h in range(3):
                            nc.sync.dma_start(
                                wo_sb[h], wo_d[h * 64:(h + 1) * 64, :])
                    for mc in range(5):
                        mw = 128 if mc < 4 else 64
                        for half in range(2):
                            c0 = half * 256
                            psb = projpsum.tile([128, 512], F32, tag="proj",
                                                name=f"proj{tb}_{mc}_{half}")
                            ps = psb[:, c0:c0 + 256]
                            for ct in range(6):
                                nc.tensor.matmul(
                                    ps[:mw, :],
                                    lhsT=wqkv_sb[ct][:,
                                                     mc * 128:mc * 128 + mw],
                                    rhs=xts[ct][:, c0:c0 + 256],
                                    start=(ct == 0), stop=(ct == 5))
                            dest = proj_dest[mc][:, tb * 512 + c0:
                                                 tb * 512 + c0 + 256]
                            with nc.allow_low_precision(
                                    reason="bf16 activations"):
                                nc.vector.tensor_scalar_add(
                                    dest, ps[:mw, :], bias_sb[mc][:mw, :])

                    # ---- v natural transpose for this block ----
                    # vt double-buffers inside the bf16 tail of the ypvt
                    # bank (cols 768:896 of the bf16 view).
                    for h in range(3):
                        for i in range(4):
                            ck = tb * 4 + i
                            ypvt = miscpsum.tile([128, 448], F32, tag="ypvt",
                                                 name=f"ypvt_vt{h}_{ck}")
                            vt16 = ypvt.bitcast(BF16)
                            half = 768 + 64 * ((h * 4 + i) % 2)
                            tp2 = vt16[:, half:half + 64]
                            nc.tensor.transpose(
                                tp2,
                                v_src[h][:, ck * 128:(ck + 1) * 128],
                                v_idn[h])
                            with nc.allow_low_precision(reason="bf16 v"):
                                nc.vector.tensor_copy(
                                    vs[h][:, ck * 65:ck * 65 + 64], tp2)

                    # ---- attention for q blocks 2*tb and 2*tb+1 ----
                    for qb in (2 * tb, 2 * tb + 1):
                        q_sl = slice(qb * 256, (qb + 1) * 256)
                        attn = [apool.tile([64, 256], BF16, tag=f"attn{h}",
                                           name=f"attn{h}_{qb}")
                                for h in range(3)]
                        kbn = 2 * qb + 2
                        ngroups = (kbn + 3) // 4
                        accbc = {}

                        def s_group(h, g, gk):
                            sp = spsum.tile([128, gk * 256], F32, tag="s",
                                            name=f"s{qb}_{h}_{g}")
                            pt = ppool.tile([128, gk * 256], BF16,
                                            tag="p", name=f"p{qb}_{h}_{g}")
                            for j in range(gk):
                                kbi = g * 4 + j
                                diag = (kbi == 2 * qb or kbi == 2 * qb + 1)
                                nc.tensor.matmul(
                                    sp[:, j * 256:(j + 1) * 256],
                                    lhsT=k_ap(h)[:, kbi * 128:
                                                 (kbi + 1) * 128],
                                    rhs=q_ap(h)[:, q_sl],
                                    start=True, stop=not diag)
                                if diag:
                                    m_sl = (slice(128, 384) if kbi == 2 * qb
                                            else slice(0, 256))
                                    nc.tensor.matmul(
                                        sp[:, j * 256:(j + 1) * 256],
                                        lhsT=ident,
                                        rhs=bigadd[:, m_sl],
                                        start=False, stop=True)
                            with nc.allow_low_precision(
                                    reason="bf16 softmax weights"):
                                nc.scalar.activation(pt, sp, Act.Exp,
                                                     bias=0.0, scale=0.125)
                            return pt

                        def pv_group(h, g, gk, pt):
                            acc = accbc[h][0:65, 0:256]
                            for j in range(gk):
                                kbi = g * 4 + j
                                nc.tensor.matmul(
                                    acc,
                                    lhsT=vs[h][:, kbi * 65:kbi * 65 + 65],
                                    rhs=pt[:, j * 256:(j + 1) * 256],
                                    start=(kbi == 0),
                                    stop=(kbi == kbn - 1))

                        def epilogue(h):
                            # Copy acc out of PSUM (DVE may read only one
                            # PSUM operand per op), take 1/denom on the row
                            # at partition 64, and broadcast it across
                            # partitions 0:64 with a K=1 matmul whose
                            # operands both live at base partition 64.
                            acc = accbc[h][0:65, 0:256]
                            acc_sb = rpool.tile([65, 256], F32, tag="accsb",
                                                name=f"accsb{qb}_{h}")
                            nc.vector.tensor_copy(acc_sb, acc)
                            rec = rpool.tile([65, 256], F32R, tag="rec",
                                             name=f"rec{qb}_{h}")
                            with nc.allow_low_precision(
                                    reason="fp32r operand rounding"):
                                nc.vector.reciprocal(rec[64:65, :],
                                                     acc_sb[64:65, :])
                            bc = accbc[h][0:64, 256:512]
                            nc.tensor.matmul(bc, lhsT=ones65[64:65, :],
                                             rhs=rec[64:65, :],
                                             start=True, stop=True)
                            with nc.allow_low_precision(
                                    reason="bf16 attn values"):
                                nc.vector.tensor_mul(attn[h],
                                                     acc_sb[0:64, :], bc)

                        # heads 0,1 interleaved per group: while ACT runs
                        # exp for one head, PE streams the other head's
                        # matmuls, hiding the exp latency.
                        for h in (0, 1):
                            accbc[h] = miscpsum.tile(
                                [128, 512], F32, tag=f"accbc{h}",
                                name=f"accbc{qb}_{h}")
                        for g in range(ngroups):
                            gk = min(4, kbn - g * 4)
                            pts = [s_group(h, g, gk) for h in (0, 1)]
                            for h in (0, 1):
                                pv_group(h, g, gk, pts[h])
                        for h in (0, 1):
                            epilogue(h)
                        accbc[2] = miscpsum.tile([128, 512], F32,
                                                 tag="accbc0",
                                                 name=f"accbc{qb}_2")
                        for g in range(ngroups):
                            gk = min(4, kbn - g * 4)
                            pv_group(2, g, gk, s_group(2, g, gk))
                        epilogue(2)
                        # out-projection for this q block
                        for mt in range(2):
                            t_sl = slice(mt * 128, (mt + 1) * 128)
                            row0 = qb * 256 + mt * 128
                            for n0, nw in ((0, 384), (384, 384)):
                                ypvt = miscpsum.tile([128, 448], F32,
                                                     tag="ypvt",
                                                     name=f"y{qb}_{mt}_{n0}")
                                yp = ypvt[:, 0:384]
                                for h in range(3):
                                    nc.tensor.matmul(
                                        yp,
                                        lhsT=attn[h][:, t_sl],
                                        rhs=wo_sb[h][:, n0:n0 + nw],
                                        start=(h == 0), stop=(h == 2))
                                ys = ysb.tile([128, 384], F32, tag="ys",
                                              name=f"ys{qb}_{mt}_{n0}")
                                nc.vector.tensor_copy(ys, yp)
                                nc.sync.dma_start(
                                    y_d[row0:row0 + 128, n0:n0 + nw], ys)
    nc.compile()
    return nc


_PROG_CACHE = {}


def _get_program(t=T):
    if t not in _PROG_CACHE:
        _PROG_CACHE[t] = build_program(t)
    return _PROG_CACHE[t]


def make_in_maps(x, wq, bq, wk, bk, wv, bv, wo):
    bf16 = mybir.dt.np(BF16)
    in_maps = []
    for core in range(NCORES):
        b = core // 4
        hs = (core % 4) * HPC
        sl = [slice((hs + h) * DH, (hs + h + 1) * DH) for h in range(HPC)]
        # columns: q0,q1 | k0,k1 | q2,v0 | k2,v1 | v2
        cols = [wq[sl[0]].T, wq[sl[1]].T, wk[sl[0]].T, wk[sl[1]].T,
                wq[sl[2]].T, wv[sl[0]].T, wk[sl[2]].T, wv[sl[1]].T,
                wv[sl[2]].T]
        biases = [bq[sl[0]], bq[sl[1]], bk[sl[0]], bk[sl[1]],
                  bq[sl[2]], bv[sl[0]], bk[sl[2]], bv[sl[1]], bv[sl[2]]]
        wqkvT = np.ascontiguousarray(
            np.concatenate(cols, axis=1)).astype(bf16)
        bqkv = np.ascontiguousarray(
            np.concatenate(biases)[:, None], dtype=np.float32)
        ch = slice(hs * DH, (hs + HPC) * DH)
        woT = np.ascontiguousarray(wo[:, ch].T).astype(bf16)
        xT = np.ascontiguousarray(x[b].T).astype(bf16)
        in_maps.append({
            "xT": xT,
            "wqkvT": wqkvT,
            "bqkv": bqkv,
            "woT": woT,
        })
    return in_maps


def run(inputs, t=T, trace=False, **kw):
    """Run on hardware; returns (y, BassKernelResults)."""
    arrs = {k: np.asarray(v, dtype=np.float32) for k, v in inputs.items()}
    nc = _get_program(t)
    in_maps = make_in_maps(**arrs)
    res = run_bass_kernel_spmd(nc, in_maps, list(range(NCORES)),
                               trace=trace, **kw)
    outs = [np.asarray(m["y"], dtype=np.float32) for m in res.results]
    y = np.empty((B, t, EMBED), dtype=np.float32)
    for b in range(B):
        y[b] = outs[4 * b] + outs[4 * b + 1] + outs[4 * b + 2] + outs[4 * b + 3]
    return y, res


def kernel(**inputs):
    y, _ = run(inputs)
    return y


# revision 18
# speedup vs baseline: 1.7695x; 1.7695x over previous
"""Causal self-attention on 8 Trainium2 NeuronCores.

Sharding: B*H = 2*12 = 24 (batch, head) pairs -> 3 heads per core.
Core i handles batch i//4, heads 3*(i%4) .. 3*(i%4)+2.
Each core computes q/k/v projections for its 3 heads (tensor-parallel slice
of wq/wk/wv), causal attention, and a partial out-projection against its
192 columns of wo. Host sums the 4 partials per batch (the "all-reduce").

v2 (vs. the original baseline):
  - x is passed PRE-TRANSPOSED (xT [768, T]) and in bf16: no PE transposes
    or PSUM->SBUF copies for x, and half the DMA bytes.
  - All matmul operands are bf16 (PSUM stays fp32): same PE rate as fp32r
    at free-dim>=256, but halves SBUF traffic and enables fast DVE modes.
  - Projections, v-transpose and attention are pipelined per 512-row block
    (attention for q-block qb needs k/v rows < 256*qb+256, which is exactly
    what the projection of the same block provides).
  - Causal mask folded into the S matmul accumulation (additive -240
    triangle via lhsT=identity), so exp output feeds PV directly with no
    cross-engine mask hop; the strictly-above-diagonal kblock is emitted
    128 wide (its left half is fully masked).
  - All 48 (qb, head) chains run as a flat software pipeline two at a
    time: one chain's S/PV matmuls hide the other chain's exp latency.
  - Engine rebalance: projection bias-add epilogue on DVE (tensor_scalar);
    softmax epilogue: DVE reciprocal + PE K=1 broadcast matmul.

Per-core kernel:
  - qT/kT computed in [64, T] layout; v computed via vT then PE-transposed
    to natural [T, 64] with a ones column appended (for softmax denominators).
  - S_T[kblock, qblock] = K_blk @ Q_blk.T  (contraction d=64)
  - P_T = exp((S_T - 240*mask) / 8)   on ACT, grouped over <=4 kblocks
  - attnU_T[65, TQ] += Vaug_blk.T @ P_T   (row 64 = softmax denominator)
  - divide via reciprocal + partition-broadcast matmul, then
    y[T,768] partial = attnT.T @ woT_slice.
No max-subtraction in softmax: logits here have |.| <~ 2, exp is safe.

Partition-base alignment: per-head pairs live at the same partition offset:
  q01 [128,T] = qT_h0 (rows 0:64) | qT_h1 (rows 64:128)
  k01 [128,T] = kT_h0 | kT_h1
  qv0 [128,T] = qT_h2 | vT_h0
  kv1 [128,T] = kT_h2 | vT_h1
  v2t [64,T]  = vT_h2
"""

import numpy as np

import concourse.mybir as mybir
from concourse import bacc
from concourse import tile
from concourse.bass_utils import run_bass_kernel_spmd
from concourse.masks import make_identity

F32 = mybir.dt.float32
F32R = mybir.dt.float32r
BF16 = mybir.dt.bfloat16

EMBED = 768
NHEAD = 12
DH = 64
B = 2
T = 4096
HPC = 3          # heads per core
CH = HPC * DH    # 192 channels per core
NCORES = 8


def build_program(t=T):
    """Build the single-core SPMD Bass program."""
    nqb = t // 256   # q blocks of 256
    ntb = t // 512   # projection T-blocks of 512

    nc = bacc.Bacc("TRN2", target_bir_lowering=False, debug=False,
                   num_devices=NCORES)

    xT_d = nc.dram_tensor("xT", [EMBED, t], BF16, kind="ExternalInput")
    # columns: q0,q1 | k0,k1 | q2,v0 | k2,v1 | v2   (64 each)
    wqkv_d = nc.dram_tensor("wqkvT", [EMBED, 576], BF16, kind="ExternalInput")
    bqkv_d = nc.dram_tensor("bqkv", [576, 1], F32, kind="ExternalInput")
    wo_d = nc.dram_tensor("woT", [CH, EMBED], BF16, kind="ExternalInput")
    y_d = nc.dram_tensor("y", [t, EMBED], F32, kind="ExternalOutput")

    Act = mybir.ActivationFunctionType

    with tile.TileContext(nc) as tc:
        with (
            tc.tile_pool(name="const", bufs=1) as cpool,
            tc.tile_pool(name="persist", bufs=1) as perm,
        ):
            ident = cpool.tile([128, 128], BF16, tag="ident")
            make_identity(nc, ident)
            # bigadd[si, u] = -240.0 if si > u - 128 else 0.0 (causal mask,
            # added to logits inside the S matmul accumulation group via
            # lhsT=identity: exp(0.125*(s-240)) ~ 9e-14 ~ 0).
            # diag kblock (d=0)  -> slice [:, 128:384];  d=-128 -> [:, 0:256]
            bigadd = cpool.tile([128, 384], BF16, tag="bigadd")
            nc.gpsimd.memset(bigadd, -240.0)
            nc.gpsimd.affine_select(
                out=bigadd, in_=bigadd,
                compare_op=mybir.AluOpType.is_ge, fill=0.0,
                base=127, pattern=[[-1, 384]], channel_multiplier=1,
            )

            # weights (tiles only; DMAs are emitted interleaved with the
            # first x-tile loads inside the tb=0 iteration)
            wqkv_sb = [cpool.tile([128, 576], BF16, name=f"wqkv{kt}",
                                  tag=f"wqkv{kt}") for kt in range(6)]
            bias_sb = [cpool.tile([128, 1], F32, name=f"bias{mc}",
                                  tag=f"bias{mc}") for mc in range(5)]
            wo_sb = [cpool.tile([64, EMBED], BF16, name=f"wo{h}",
                                tag=f"wo{h}") for h in range(3)]

            # persistent activations (bf16)
            q01 = perm.tile([128, t], BF16, tag="q01")
            k01 = perm.tile([128, t], BF16, tag="k01")
            qv0 = perm.tile([128, t], BF16, tag="qv0")
            kv1 = perm.tile([128, t], BF16, tag="kv1")
            v2t = perm.tile([64, t], BF16, tag="v2t")
            # all-ones row at partition 64 (for denominator broadcast mm)
            ones65 = cpool.tile([65, 64], F32R, tag="ones65")
            nc.gpsimd.memset(ones65.bitcast(F32), 1.0)
            # v natural, 65-wide per 128-row chunk (col 64 = ones)
            vs = [perm.tile([128, (t // 128) * 65], BF16, name=f"vs{h}",
                            tag=f"vs{h}")
                  for h in range(3)]
            for h in range(3):
                nc.gpsimd.memset(vs[h], 1.0)

            proj_dest = [q01, k01, qv0, kv1, v2t]

            def q_ap(h):
                return (q01[0:64], q01[64:128], qv0[0:64])[h]

            def k_ap(h):
                return (k01[0:64], k01[64:128], kv1[0:64])[h]

            v_src = [qv0[64:128], kv1[64:128], v2t[0:64]]
            v_idn = [ident[64:128, 64:128], ident[64:128, 64:128],
                     ident[0:64, 0:64]]

            with (
                tc.tile_pool(name="xtpool", bufs=3) as xtpool,
                tc.tile_pool(name="projpsum", bufs=1, space="PSUM") as projpsum,
                tc.tile_pool(name="spsum", bufs=2, space="PSUM") as spsum,
                tc.tile_pool(name="miscpsum", bufs=1, space="PSUM") as miscpsum,
                tc.tile_pool(name="ppool", bufs=4) as ppool,
                tc.tile_pool(name="apool", bufs=3) as apool,
                tc.tile_pool(name="rpool", bufs=4) as rpool,
                tc.tile_pool(name="ysb", bufs=4) as ysb,
            ):
                xts_of = {}

                def prefetch_x(tb):
                    """Issue the x-tile DMAs for block tb (one block of
                    lead time ahead of the matmuls that consume them)."""
                    if tb >= ntb or tb in xts_of:
                        return
                    xts = [xtpool.tile([128, 512], BF16, tag=f"xt{ct}",
                                       name=f"xt{ct}_{tb}")
                           for ct in range(6)]
                    for ct in range(6):
                        if tb == 0:
                            nc.sync.dma_start(
                                wqkv_sb[ct],
                                wqkv_d[ct * 128:(ct + 1) * 128, :])
                        nc.sync.dma_start(
                            xts[ct],
                            xT_d[ct * 128:(ct + 1) * 128,
                                 tb * 512:(tb + 1) * 512])
                    if tb == 0:
                        for mc in range(5):
                            mw = 128 if mc < 4 else 64
                            nc.sync.dma_start(
                                bias_sb[mc][:mw, :],
                                bqkv_d[mc * 128:mc * 128 + mw, :])
                        for h in range(3):
                            nc.sync.dma_start(
                                wo_sb[h], wo_d[h * 64:(h + 1) * 64, :])
                    xts_of[tb] = xts

                def emit_proj(tb):
                    """Projections + v-transpose for rows
                    [tb*512, (tb+1)*512)."""
                    prefetch_x(tb)
                    xts = xts_of.pop(tb)
                    prefetch_x(tb + 1)
                    for mc in range(5):
                        mw = 128 if mc < 4 else 64
                        ps = projpsum.tile([128, 512], F32, tag="proj",
                                           name=f"proj{tb}_{mc}")
                        for ct in range(6):
                            nc.tensor.matmul(
                                ps[:mw, :],
                                lhsT=wqkv_sb[ct][:, mc * 128:mc * 128 + mw],
                                rhs=xts[ct],
                                start=(ct == 0), stop=(ct == 5))
                        dest = proj_dest[mc][:, tb * 512:(tb + 1) * 512]
                        with nc.allow_low_precision(
                                reason="bf16 activations"):
                            nc.vector.tensor_scalar_add(
                                dest, ps[:mw, :], bias_sb[mc][:mw, :])
                    # v natural transpose (vt double-buffers inside the
                    # bf16 tail of the ypvt bank, cols 768:896 of the bf16
                    # view).
                    for h in range(3):
                        for i in range(4):
                            ck = tb * 4 + i
                            ypvt = miscpsum.tile([128, 448], F32, tag="ypvt",
                                                 name=f"ypvt_vt{h}_{ck}")
                            vt16 = ypvt.bitcast(BF16)
                            half = 768 + 64 * ((h * 4 + i) % 2)
                            tp2 = vt16[:, half:half + 64]
                            nc.tensor.transpose(
                                tp2,
                                v_src[h][:, ck * 128:(ck + 1) * 128],
                                v_idn[h])
                            with nc.allow_low_precision(reason="bf16 v"):
                                nc.vector.tensor_copy(
                                    vs[h][:, ck * 65:ck * 65 + 64], tp2)

                def group_blocks(qb, g):
                    """Block layout of kblock group g for q-block qb:
                    (kbi, sp col offset, width, q col offset).  The
                    strictly-above-diagonal kblock (kbi == 2*qb+1) only
                    touches q columns 128:256, so it is emitted 128 wide."""
                    kbn = 2 * qb + 2
                    gk = min(4, kbn - g * 4)
                    blocks, c0 = [], 0
                    for j in range(gk):
                        kbi = g * 4 + j
                        w = 128 if kbi == 2 * qb + 1 else 256
                        q0 = 128 if kbi == 2 * qb + 1 else 0
                        blocks.append((kbi, c0, w, q0))
                        c0 += w
                    return blocks, c0

                def s_group(qb, h, g):
                    """S matmuls + exp for kblock group g of chain (qb,h).
                    The causal mask is folded into the S accumulation as an
                    additive -240 triangle via lhsT=identity."""
                    blocks, total = group_blocks(qb, g)
                    sp = spsum.tile([128, total], F32, tag="s",
                                    name=f"s{qb}_{h}_{g}")
                    pt = ppool.tile([128, total], BF16,
                                    tag="p", name=f"p{qb}_{h}_{g}")
                    for kbi, c0, w, q0 in blocks:
                        diag = (kbi == 2 * qb or kbi == 2 * qb + 1)
                        nc.tensor.matmul(
                            sp[:, c0:c0 + w],
                            lhsT=k_ap(h)[:, kbi * 128:(kbi + 1) * 128],
                            rhs=q_ap(h)[:, qb * 256 + q0:qb * 256 + q0 + w],
                            start=True, stop=not diag)
                        if diag:
                            m0 = 128
                            nc.tensor.matmul(
                                sp[:, c0:c0 + w],
                                lhsT=ident,
                                rhs=bigadd[:, m0:m0 + w],
                                start=False, stop=True)
                    with nc.allow_low_precision(
                            reason="bf16 softmax weights"):
                        nc.scalar.activation(pt, sp, Act.Exp,
                                             bias=0.0, scale=0.125)
                    return pt

                def pv_group(qb, h, g, pt, accbc):
                    """PV accumulation.  acc columns 0:128 close (stop) at
                    the kbi==2*qb block's left half; columns 128:256 close
                    at the 128-wide kbi==2*qb+1 block."""
                    kbn = 2 * qb + 2
                    blocks, _ = group_blocks(qb, g)
                    acc = accbc[0:65, 0:256]
                    for kbi, c0, w, q0 in blocks:
                        vsl = vs[h][:, kbi * 65:kbi * 65 + 65]
                        if kbi == 2 * qb:
                            nc.tensor.matmul(
                                acc[:, 0:128], lhsT=vsl,
                                rhs=pt[:, c0:c0 + 128],
                                start=(kbi == 0), stop=True)
                            nc.tensor.matmul(
                                acc[:, 128:256], lhsT=vsl,
                                rhs=pt[:, c0 + 128:c0 + 256],
                                start=(kbi == 0), stop=False)
                        elif kbi == 2 * qb + 1:
                            nc.tensor.matmul(
                                acc[:, 128:256], lhsT=vsl,
                                rhs=pt[:, c0:c0 + 128],
                                start=False, stop=True)
                        else:
                            nc.tensor.matmul(
                                acc, lhsT=vsl,
                                rhs=pt[:, c0:c0 + 256],
                                start=(kbi == 0), stop=False)

                def epilogue(qb, h, accbc, attn_t):
                    """Copy acc out of PSUM (DVE may read only one PSUM
                    operand per op), take 1/denom on the row at partition
                    64, and broadcast it across partitions 0:64 with a K=1
                    matmul whose operands both live at base partition 64."""
                    acc = accbc[0:65, 0:256]
                    acc_sb = rpool.tile([65, 256], F32, tag="accsb",
                                        name=f"accsb{qb}_{h}")
                    nc.vector.tensor_copy(acc_sb, acc)
                    rec = rpool.tile([65, 256], F32R, tag="rec",
                                     name=f"rec{qb}_{h}")
                    with nc.allow_low_precision(
                            reason="fp32r operand rounding"):
                        nc.vector.reciprocal(rec[64:65, :],
                                             acc_sb[64:65, :])
                    bc = accbc[0:64, 256:512]
                    nc.tensor.matmul(bc, lhsT=ones65[64:65, :],
                                     rhs=rec[64:65, :],
                                     start=True, stop=True)
                    with nc.allow_low_precision(reason="bf16 attn values"):
                        nc.vector.tensor_mul(attn_t, acc_sb[0:64, :], bc)

                def emit_outproj(qb, attn_of):
                    for mt in range(2):
                        t_sl = slice(mt * 128, (mt + 1) * 128)
                        row0 = qb * 256 + mt * 128
                        for n0, nw in ((0, 384), (384, 384)):
                            ypvt = miscpsum.tile([128, 448], F32,
                                                 tag="ypvt",
                                                 name=f"y{qb}_{mt}_{n0}")
                            yp = ypvt[:, 0:384]
                            for h in range(3):
                                nc.tensor.matmul(
                                    yp,
                                    lhsT=attn_of[h][:, t_sl],
                                    rhs=wo_sb[h][:, n0:n0 + nw],
                                    start=(h == 0), stop=(h == 2))
                            ys = ysb.tile([128, 384], F32, tag="ys",
                                          name=f"ys{qb}_{mt}_{n0}")
                            nc.vector.tensor_copy(ys, yp)
                            nc.sync.dma_start(
                                y_d[row0:row0 + 128, n0:n0 + nw], ys)

                # ---- flat chain pipeline: all (qb, h) chains processed
                # two at a time so one chain's PE matmuls hide the other's
                # exp latency; projections are emitted on demand.
                chains = [(qb, h) for qb in range(nqb) for h in range(3)]
                attn_of = {}
                done_proj = -1
                for ci in range(0, len(chains), 2):
                    pair = chains[ci:ci + 2]
                    for qb, h in pair:
                        while done_proj < qb // 2:
                            done_proj += 1
                            emit_proj(done_proj)
                    accs = []
                    for idx, (qb, h) in enumerate(pair):
                        accs.append(miscpsum.tile(
                            [128, 512], F32, tag=f"accbc{(ci + idx) % 2}",
                            name=f"accbc{qb}_{h}"))
                        attn_of.setdefault(qb, {})[h] = apool.tile(
                            [64, 256], BF16, tag=f"attn{h}",
                            name=f"attn{h}_{qb}")
                    ngs = [(2 * qb + 2 + 3) // 4 for qb, h in pair]
                    for g in range(max(ngs)):
                        pts = [s_group(qb, h, g) if g < ngs[idx] else None
                               for idx, (qb, h) in enumerate(pair)]
                        for idx, (qb, h) in enumerate(pair):
                            if pts[idx] is not None:
                                pv_group(qb, h, g, pts[idx], accs[idx])
                    for idx, (qb, h) in enumerate(pair):
                        epilogue(qb, h, accs[idx], attn_of[qb][h])
                        if h == 2:
                            emit_outproj(qb, attn_of.pop(qb))
    nc.compile()
    return nc


_PROG_CACHE = {}


def _get_program(t=T):
    if t not in _PROG_CACHE:
        _PROG_CACHE[t] = build_program(t)
    return _PROG_CACHE[t]


def make_in_maps(x, wq, bq, wk, bk, wv, bv, wo):
    bf16 = mybir.dt.np(BF16)
    in_maps = []
    for core in range(NCORES):
        b = core // 4
        hs = (core % 4) * HPC
        sl = [slice((hs + h) * DH, (hs + h + 1) * DH) for h in range(HPC)]
        # columns: q0,q1 | k0,k1 | q2,v0 | k2,v1 | v2
        cols = [wq[sl[0]].T, wq[sl[1]].T, wk[sl[0]].T, wk[sl[1]].T,
                wq[sl[2]].T, wv[sl[0]].T, wk[sl[2]].T, wv[sl[1]].T,
                wv[sl[2]].T]
        biases = [bq[sl[0]], bq[sl[1]], bk[sl[0]], bk[sl[1]],
                  bq[sl[2]], bv[sl[0]], bk[sl[2]], bv[sl[1]], bv[sl[2]]]
        wqkvT = np.ascontiguousarray(
            np.concatenate(cols, axis=1)).astype(bf16)
        bqkv = np.ascontiguousarray(
            np.concatenate(biases)[:, None], dtype=np.float32)
        ch = slice(hs * DH, (hs + HPC) * DH)
        woT = np.ascontiguousarray(wo[:, ch].T).astype(bf16)
        xT = np.ascontiguousarray(x[b].T).astype(bf16)
        in_maps.append({
            "xT": xT,
            "wqkvT": wqkvT,
            "bqkv": bqkv,
            "woT": woT,
        })
    return in_maps


def run(inputs, t=T, trace=False, **kw):
    """Run on hardware; returns (y, BassKernelResults)."""
    arrs = {k: np.asarray(v, dtype=np.float32) for k, v in inputs.items()}
    nc = _get_program(t)
    in_maps = make_in_maps(**arrs)
    res = run_bass_kernel_spmd(nc, in_maps, list(range(NCORES)),
                               trace=trace, **kw)
    outs = [np.asarray(m["y"], dtype=np.float32) for m in res.results]
    y = np.empty((B, t, EMBED), dtype=np.float32)
    for b in range(B):
        y[b] = outs[4 * b] + outs[4 * b + 1] + outs[4 * b + 2] + outs[4 * b + 3]
    return y, res


def kernel(**inputs):
    y, _ = run(inputs)
    return y
